# revision 6
# baseline (speedup 1.0000x reference)
"""Trainium2 Bass kernel for nn_Decoder (mean-pool L=16 + overlap-add step 8).

Math (per (b, c) slice, est = est_source[b, c] of shape [256, 4000]):
  A[g, f]      = (1/16) * sum_{l=0..15} est[16*g + l, f]          g in 0..15
  out[8*s + j] = A[j, s] + A[8+j, s-1]                            s in 0..4000
with A[., -1] = A[., 4000] = 0 at the edges.  Output length 8*4001 = 32008.

Kernel strategy (8 cores, 4 slices each): every output sample is the sum of
32 input values (16 from the low half at frame s, 16 from the high half at
frame s-1).  The host packs those 32 values contiguously along the free dim
(pure gather/layout; the 1/16 is folded into the fp32->fp16 cast as an exact
exponent shift), with the partition dim mapping to contiguous blocks of 32
output frames, so on device each chunk is just:

  HWDGE load [128, 2048] fp16  ->  DVE tensor_reduce(add, axis=X) over
  [128, 64, 32] -> [128, 64] fp32  ->  HWDGE store (scalar queue)

Partition p of slice i holds output samples [256*p, 256*(p+1)) of y[i], so
the store is one fully contiguous 128 KiB DMA per slice.  No matmul, no
PSUM, no gpsimd: loads issue on the sync HWDGE queue, stores on the scalar
HWDGE queue, and the only compute engine is the DVE (~8.6 us of reduces,
hidden under ~24 us of loads).
"""

import sys

if "/opt/trn_rl_repo" not in sys.path:
    sys.path.insert(0, "/opt/trn_rl_repo")

import numpy as np


def _install_ntff_hook():
    """Provide antenv.axon_hooks (absent in this image) so trace=True works.

    The boot-side installer (trn_agent_boot.trn_boot) skips hook setup when
    antenv.axon_hooks is missing; bass_utils then refuses to trace.  We
    register a lazy equivalent backed by the same ctypes NTFF driver.
    """
    import types
    try:
        import antenv
    except ImportError:
        return
    if "antenv.axon_hooks" in sys.modules:
        return
    mod = types.ModuleType("antenv.axon_hooks")
    _state = {}

    def set_axon_ntff_profile_hook(h):
        _state["h"] = h

    def get_axon_ntff_profile_hook():
        if "h" not in _state:
            try:
                from trn_agent_boot.trn_boot import _ntff_profile_via_ctypes
                _state["h"] = _ntff_profile_via_ctypes("/opt/axon/libaxon_pjrt.so")
            except Exception:
                _state["h"] = None
        return _state["h"]

    mod.set_axon_ntff_profile_hook = set_axon_ntff_profile_hook
    mod.get_axon_ntff_profile_hook = get_axon_ntff_profile_hook
    sys.modules["antenv.axon_hooks"] = mod
    antenv.axon_hooks = mod


_install_ntff_hook()

import concourse.bass as bass
import concourse.mybir as mybir
from concourse import tile
from concourse.bass_utils import run_bass_kernel_spmd


class _SingleWaitTileContext(tile.TileContext):
    """TileContext whose kernel-tail drain never carries multiple sem waits.

    The pinned walrus build rejects any instruction with more than one sync
    wait ("Too many sync wait commands").  Tile's default exit emits a single
    Drain waiting on every outstanding proc semaphore.  Instead, emit one
    wait_ge per proc on the SP sequencer (each a single-wait instruction),
    then a wait-free drain.
    """

    # proc indices >= _FIRST_DMA_PROC are DMA lanes whose semaphores advance
    # by 16 per op (one inc per SDMA engine) while the vector clock ticks 1.
    _FIRST_DMA_PROC = 11

    def _drain_and_barrier(self, tick_clock, wait_clock):
        nc = self.nc
        clock = tick_clock.global_clock  # bass_rust.VectorClock: 27 ints
        allocated = wait_clock.sems.allocated()
        for proc_idx, tick in enumerate(clock):
            if tick > 0 and proc_idx in allocated:
                val = tick * 16 if proc_idx >= self._FIRST_DMA_PROC else tick
                nc.sync.wait_ge(allocated[proc_idx], val)
        nc.sync.drain()
        nc.all_engine_barrier()
        popped = nc._tile_sem_poison_stack.pop()
        assert popped is self._sem_poison
        nc.clear_and_free_semaphores(list(self.sems.allocated().values()))
        nc.all_engine_barrier()

# Problem constants (hardcoded per spec)
B, C, D2, FRAMES = 16, 2, 256, 4000
L = 16
SUB = FRAMES + 1          # 4001 output subframes per slice
OUT_LEN = 8 * SUB         # 32008
N_CORES = 8
SLICES = (B * C) // N_CORES   # 4 slices per core
FPP = 32                  # output frames per partition (128*32 = 4096 >= 4001)
PADF = 128 * FPP          # 4096 padded frames
INC = 8 * 32 * FPP        # 8192 fp16 input cols per partition per slice
OUTC = 8 * FPP            # 256 fp32 output cols per partition per slice
NCH = 4                   # load/reduce chunks per slice
CCOL = INC // NCH         # 2048 input cols per chunk
OCH = OUTC // NCH         # 64 output cols per chunk

_CACHE = {}


def _build_nc() -> bass.Bass:
    f16 = mybir.dt.float16
    f32 = mybir.dt.float32

    nc = bass.Bass()
    # Host-packed input: x[i, c, p, :] = chunk c of slice i, partition p.
    # Free layout per partition is (t, j, m): t = frame-within-partition,
    # j = output phase, m = the 32 fp16 addends of output sample
    # 256*p + 8*t + j (already scaled by 1/16 on the host).
    x_d = nc.dram_tensor("x", [SLICES, NCH, 128, CCOL], f16,
                         kind="ExternalInput")
    # y[i, p, 8*t + j] = output sample 256*p + 8*t + j of slice i
    # (so y[i] viewed flat is the first 32768 samples, host trims to 32008).
    y_d = nc.dram_tensor("y", [SLICES, 128, OUTC], f32, kind="ExternalOutput")

    # Walrus allows only ONE sync wait per DMA instruction.  Tile spreads
    # HWDGE DMAs over 8 logical queue procs and adds a queue-ordering wait
    # from the 2nd DMA per proc on, so HWDGE stores (queue wait + data
    # wait) are rejected.  Loads carry only the queue wait (their buffers
    # are fresh).  Stores go on SWDGE (gpsimd) procs instead: with <= 8
    # stores each lands at tick 1 (no queue wait) and carries a single
    # merged DVE wait.  gpsimd is otherwise idle, so its per-store
    # descriptor generation overlaps the load stream.
    with _SingleWaitTileContext(nc) as tc:
        with (
            tc.tile_pool(name="xp", bufs=SLICES * NCH) as xp,
            tc.tile_pool(name="yp", bufs=SLICES) as yp,
        ):
            for i in range(SLICES):
                yt = yp.tile([128, OUTC], f32, tag="y")
                for c in range(NCH):
                    xt = xp.tile([128, CCOL], f16, tag="x")
                    nc.sync.dma_start(out=xt[:], in_=x_d[i, c])
                    nc.vector.tensor_reduce(
                        out=yt[:, c * OCH : (c + 1) * OCH],
                        in_=xt[:].rearrange("p (k m) -> p k m", m=32),
                        axis=mybir.AxisListType.X,
                        op=mybir.AluOpType.add,
                    )
                    if c % 2 == 1:
                        # Half-slice store: [128, 128] f32, 512 B per
                        # partition contiguous in DRAM.
                        h0 = (c - 1) * OCH
                        nc.gpsimd.dma_start(
                            out=y_d[i][:, h0 : h0 + 2 * OCH],
                            in_=yt[:, h0 : h0 + 2 * OCH],
                        )
    return nc


def _get_nc():
    if "nc" not in _CACHE:
        _CACHE["nc"] = _build_nc()
    return _CACHE["nc"]


def _prep_inputs(flat: np.ndarray) -> np.ndarray:
    """Pack [S, 256, 4000] fp32 into [S, NCH, 128, CCOL] fp16 addend groups.

    Pure gather/layout; the only arithmetic is the fp32->fp16 cast with the
    exact 1/16 exponent shift folded in.
    """
    S = flat.shape[0]
    sc = (flat * np.float32(1.0 / L)).astype(np.float16)
    xl = sc[:, :128, :].reshape(S, 8, 16, FRAMES)
    xh = sc[:, 128:, :].reshape(S, 8, 16, FRAMES)
    pk = np.zeros((S, PADF, 8, 32), dtype=np.float16)
    pk[:, :FRAMES, :, :16] = xl.transpose(0, 3, 1, 2)
    pk[:, 1 : FRAMES + 1, :, 16:] = xh.transpose(0, 3, 1, 2)
    # s = 32*p + t  ->  [S, 128 p, INC] -> chunk-major [S, NCH, 128, CCOL]
    x = pk.reshape(S, 128, NCH, CCOL).transpose(0, 2, 1, 3)
    return np.ascontiguousarray(x)


def kernel(est_source: np.ndarray, _trace: bool = False) -> np.ndarray:
    est = np.ascontiguousarray(np.asarray(est_source), dtype=np.float32)
    assert est.shape == (B, C, D2, FRAMES)
    flat = est.reshape(B * C, D2, FRAMES)
    x = _prep_inputs(flat)

    nc = _get_nc()
    in_maps = [
        {"x": x[SLICES * k : SLICES * (k + 1)]}
        for k in range(N_CORES)
    ]
    res = run_bass_kernel_spmd(nc, in_maps, core_ids=list(range(N_CORES)),
                               trace=_trace)
    _CACHE["last_results"] = res
    outs = [
        res.results[k]["y"].reshape(SLICES, 128 * OUTC)[:, :OUT_LEN]
        for k in range(N_CORES)
    ]
    return np.concatenate(outs, axis=0).reshape(B, C, OUT_LEN)


# revision 8
# speedup vs baseline: 1.1714x; 1.1714x over previous
"""Trainium2 Bass kernel for nn_Decoder (mean-pool L=16 + overlap-add step 8).

Math (per (b, c) slice, est = est_source[b, c] of shape [256, 4000]):
  A[g, f]      = (1/16) * sum_{l=0..15} est[16*g + l, f]          g in 0..15
  out[8*s + j] = A[j, s] + A[8+j, s-1]                            s in 0..4000
with A[., -1] = A[., 4000] = 0 at the edges.  Output length 8*4001 = 32008.

Kernel strategy (8 cores, 4 slices each): every output sample is the sum of
32 input values (16 from the low half at frame s, 16 from the high half at
frame s-1).  The host packs those 32 addends *m-major* per chunk (addend m
of output k at column m*K + k; pure gather/layout -- the 1/16 is folded
into the fp32->fp16 cast as an exact exponent shift), with the partition
dim mapping to contiguous blocks of output samples.  On device each chunk
is then a binary reduction tree of five tensor_tensor adds, each summing
the first contiguous half-block with the second:

  [128, 4096] f16 -> 2048 -> 1024 -> 512 -> 256 -> [128, 128] f32

Levels 1-4 keep f16 operands packed (DVE 2x mode, 0.5 cyc/elem); the last
level emits f32.  This beats tensor_reduce (no DVE fast modes, 1 cyc/elem)
by ~2x.  No matmul, no PSUM; the only compute engine is the DVE.

DMA structure: loads are chained head-to-tail (each load's first written
SBUF column overlaps the previous chunk's scratch column, a WAW dep) so
chunks complete in sequence at full HBM bandwidth and the reduce tree
pipelines behind the load stream -- 8 concurrent tick-1 DMAs would
otherwise all finish together and serialize all compute after them.
Stores go per chunk on SWDGE (gpsimd): walrus allows only one sync wait
per DMA instruction, and tile adds a queue-ordering wait to HWDGE DMAs
from the 2nd use of each of the 8 queue procs on; with 8 stores on 8
otherwise-idle SWDGE procs each store is tick-1 and carries just its
single DVE data wait.  Each store is [128, 128] f32, 512 B per partition,
contiguous in DRAM.
"""

import sys

if "/opt/trn_rl_repo" not in sys.path:
    sys.path.insert(0, "/opt/trn_rl_repo")

import numpy as np


def _install_ntff_hook():
    """Provide antenv.axon_hooks (absent in this image) so trace=True works.

    The boot-side installer (trn_agent_boot.trn_boot) skips hook setup when
    antenv.axon_hooks is missing; bass_utils then refuses to trace.  We
    register a lazy equivalent backed by the same ctypes NTFF driver.
    """
    import types
    try:
        import antenv
    except ImportError:
        return
    if "antenv.axon_hooks" in sys.modules:
        return
    mod = types.ModuleType("antenv.axon_hooks")
    _state = {}

    def set_axon_ntff_profile_hook(h):
        _state["h"] = h

    def get_axon_ntff_profile_hook():
        if "h" not in _state:
            try:
                from trn_agent_boot.trn_boot import _ntff_profile_via_ctypes
                _state["h"] = _ntff_profile_via_ctypes("/opt/axon/libaxon_pjrt.so")
            except Exception:
                _state["h"] = None
        return _state["h"]

    mod.set_axon_ntff_profile_hook = set_axon_ntff_profile_hook
    mod.get_axon_ntff_profile_hook = get_axon_ntff_profile_hook
    sys.modules["antenv.axon_hooks"] = mod
    antenv.axon_hooks = mod


_install_ntff_hook()

import concourse.bass as bass
import concourse.mybir as mybir
from concourse import tile
from concourse.bass_utils import run_bass_kernel_spmd


class _SingleWaitTileContext(tile.TileContext):
    """TileContext whose kernel-tail drain never carries multiple sem waits.

    The pinned walrus build rejects any instruction with more than one sync
    wait ("Too many sync wait commands").  Tile's default exit emits a single
    Drain waiting on every outstanding proc semaphore.  Instead, emit one
    wait_ge per proc on the SP sequencer (each a single-wait instruction),
    then a wait-free drain.
    """

    # proc indices >= _FIRST_DMA_PROC are DMA lanes whose semaphores advance
    # by 16 per op (one inc per SDMA engine) while the vector clock ticks 1.
    _FIRST_DMA_PROC = 11

    def _drain_and_barrier(self, tick_clock, wait_clock):
        nc = self.nc
        clock = tick_clock.global_clock  # bass_rust.VectorClock: 27 ints
        allocated = wait_clock.sems.allocated()
        for proc_idx, tick in enumerate(clock):
            if tick > 0 and proc_idx in allocated:
                val = tick * 16 if proc_idx >= self._FIRST_DMA_PROC else tick
                nc.sync.wait_ge(allocated[proc_idx], val)
        nc.sync.drain()
        nc.all_engine_barrier()
        popped = nc._tile_sem_poison_stack.pop()
        assert popped is self._sem_poison
        nc.clear_and_free_semaphores(list(self.sems.allocated().values()))
        nc.all_engine_barrier()

# Problem constants (hardcoded per spec)
B, C, D2, FRAMES = 16, 2, 256, 4000
L = 16
SUB = FRAMES + 1          # 4001 output subframes per slice
OUT_LEN = 8 * SUB         # 32008
N_CORES = 8
SLICES = (B * C) // N_CORES   # 4 slices per core
FPP = 32                  # output frames per partition (128*32 = 4096 >= 4001)
OUTC = 8 * FPP            # 256 f32 output cols per partition per slice
NCH = 2                   # chunks per slice
KCH = OUTC // NCH         # 128 output cols per partition per chunk
CCOL = 32 * KCH           # 4096 f16 input cols per partition per chunk
NCHT = SLICES * NCH       # 8 chunks per core
CSP = CCOL + 1            # chunk region pitch in SBUF (data + scratch col)

_CACHE = {}


def _build_nc() -> bass.Bass:
    f16 = mybir.dt.float16
    f32 = mybir.dt.float32

    nc = bass.Bass()
    # Host-packed input, one row of 4097 cols per chunk: chunk 0 is
    # [4096 data | 1 dummy], chunks 1..7 are [1 dummy | 4096 data].  The
    # dummy column is what each load writes into the previous chunk's
    # scratch column to form the WAW serialization chain.
    x_d = nc.dram_tensor("x", [NCHT, 128, CSP], f16, kind="ExternalInput")
    # y[i, h, p, q] = output sample 256*p + 128*h + q of slice i.
    y_d = nc.dram_tensor("y", [SLICES, NCH, 128, KCH], f32,
                         kind="ExternalOutput")

    with _SingleWaitTileContext(nc) as tc:
        with (
            tc.tile_pool(name="xp", bufs=1) as xp,
            tc.tile_pool(name="zp", bufs=NCHT) as zp,
            tc.tile_pool(name="yp", bufs=NCHT) as yp,
        ):
            # One big input buffer; chunk c's data lives at
            # [c*CSP, c*CSP + CCOL), its scratch col at (c+1)*CSP - 1.
            xb = xp.tile([128, NCHT * CSP], f16)
            for c in range(NCHT):
                i, h = divmod(c, NCH)
                lo = c * CSP
                if c == 0:
                    nc.sync.dma_start(out=xb[:, 0:CSP], in_=x_d[0])
                else:
                    # First written col = chunk c-1's scratch -> WAW dep
                    # serializes this load after load c-1 completes.
                    nc.sync.dma_start(out=xb[:, lo - 1 : lo + CCOL],
                                      in_=x_d[c])
                # Binary tree: m-major layout makes every level a sum of
                # two contiguous half-blocks (f16 packed -> DVE 2x mode).
                z1 = zp.tile([128, CCOL // 2], f16, tag="z1")
                z2 = zp.tile([128, CCOL // 4], f16, tag="z2")
                z3 = zp.tile([128, CCOL // 8], f16, tag="z3")
                z4 = zp.tile([128, CCOL // 16], f16, tag="z4")
                yo = yp.tile([128, KCH], f32, tag="y")
                add = mybir.AluOpType.add
                w = CCOL // 2
                nc.vector.tensor_tensor(out=z1[:], in0=xb[:, lo : lo + w],
                                        in1=xb[:, lo + w : lo + 2 * w], op=add)
                nc.vector.tensor_tensor(out=z2[:], in0=z1[:, : w // 2],
                                        in1=z1[:, w // 2 :], op=add)
                nc.vector.tensor_tensor(out=z3[:], in0=z2[:, : w // 4],
                                        in1=z2[:, w // 4 :], op=add)
                nc.vector.tensor_tensor(out=z4[:], in0=z3[:, : w // 8],
                                        in1=z3[:, w // 8 :], op=add)
                nc.vector.tensor_tensor(out=yo[:], in0=z4[:, : w // 16],
                                        in1=z4[:, w // 16 :], op=add)
                # SWDGE store: tick-1 on an idle gpsimd DMA proc, so its
                # single sync wait is the DVE data dependency.
                nc.gpsimd.dma_start(out=y_d[i, h], in_=yo[:])
    return nc


def _get_nc():
    if "nc" not in _CACHE:
        _CACHE["nc"] = _build_nc()
    return _CACHE["nc"]


def _prep_inputs(flat: np.ndarray) -> np.ndarray:
    """Pack [S, 256, 4000] fp32 into [S*NCH, 128, CSP] fp16 m-major chunks.

    Pure gather/layout; the only arithmetic is the fp32->fp16 cast with the
    exact 1/16 exponent shift folded in.
    """
    S = flat.shape[0]
    sc = (flat * np.float32(1.0 / L)).astype(np.float16)
    xl = sc[:, :128, :].reshape(S, 8, 16, FRAMES)
    xh = sc[:, 128:, :].reshape(S, 8, 16, FRAMES)
    # a[slice, s, j, m] = addend m of output sample 8*s + j
    a = np.zeros((S, 128 * FPP, 8, 32), dtype=np.float16)
    a[:, :FRAMES, :, :16] = xl.transpose(0, 3, 1, 2)
    a[:, 1 : FRAMES + 1, :, 16:] = xh.transpose(0, 3, 1, 2)
    # s = 32*p + t, t = (FPP//NCH)*h + t'; chunk col = m*KCH + 8*t' + j
    tpc = FPP // NCH
    a = a.reshape(S, 128, NCH, tpc, 8, 32)        # (i, p, h, t', j, m)
    a = a.transpose(0, 2, 1, 5, 3, 4)             # (i, h, p, m, t', j)
    a = np.ascontiguousarray(a).reshape(S * NCH, 128, CCOL)
    x = np.empty((S * NCH, 128, CSP), dtype=np.float16)
    first = np.arange(0, S * NCH, NCHT)   # chunk 0 of each core's chain
    rest = np.setdiff1d(np.arange(S * NCH), first)
    x[first, :, :CCOL] = a[first]
    x[first, :, CCOL] = 0
    x[rest, :, 0] = 0
    x[rest, :, 1:] = a[rest]
    return x


def kernel(est_source: np.ndarray, _trace: bool = False) -> np.ndarray:
    est = np.ascontiguousarray(np.asarray(est_source), dtype=np.float32)
    assert est.shape == (B, C, D2, FRAMES)
    flat = est.reshape(B * C, D2, FRAMES)
    x = _prep_inputs(flat)

    nc = _get_nc()
    in_maps = [
        {"x": x[NCHT * k : NCHT * (k + 1)]}
        for k in range(N_CORES)
    ]
    res = run_bass_kernel_spmd(nc, in_maps, core_ids=list(range(N_CORES)),
                               trace=_trace)
    _CACHE["last_results"] = res
    outs = []
    for k in range(N_CORES):
        yk = res.results[k]["y"]              # [SLICES, NCH, 128, KCH]
        yk = yk.transpose(0, 2, 1, 3).reshape(SLICES, 128 * OUTC)
        outs.append(yk[:, :OUT_LEN])
    return np.concatenate(outs, axis=0).reshape(B, C, OUT_LEN)
